# revision 20
# baseline (speedup 1.0000x reference)
"""Trainium2 Bass kernel for nn_CrossAttention (batch-parallel over 8 cores).

Reference computation (per batch element b):
    x   = proj_in(input)              # 1x1 conv -> [hw, emb]
    Q   = x @ wq ;  K = ctx @ wk ; V = ctx @ wv
    att = softmax(Q K^T * emb^-0.5)
    out = att @ V                     # [hw, emb]
    out = proj_out(concat([input, out], ch))   # 1x1 conv -> [in_ch, h, w]

Device strategy (data-parallel, one batch element per NeuronCore):
  * Host folds proj_in into the Q projection and fuses it into the scores:
        att^T = G^T A,  G = H^T ctx,  H = wk Wq_eff^T,
        Wq_eff = proj_in_w.T wq * emb^-0.5
    wv and the attention half of proj_out fold into one matrix W_VO, so
        out = WoA^T A + (ctx^T W_VO)^T softmax-weights.
  * The attention path (G, VV, scores, att @ V) runs in fp8e4m3 with
    DoubleRow matmuls (two 128-row k-tiles per instruction, 2x PE
    throughput).  Its contribution to the output is ~75x smaller than
    the direct WoA^T A path, so fp8 error there is negligible; the
    direct path stays bf16.  Power-of-two scales keep fp8 operands in
    the normal range: H is scaled by 1024 (undone by exp(x/1024) via the
    activation scale), W_VO by 16 (undone by a 16.0 "ones" matrix in the
    softmax-normalizer matmul, so rb = 1/(16*colsum)).
  * Per 512-position block: ST = G^T A8 (8 psum groups, 2 DR matmuls
    each) -> exp -> PT fp8; OT = VV^T PT issued PAIR-major (md inner) so
    each OT pair consumes exp tiles right as the scalar engine finishes
    them; the previous block's bf16 OUT_A = WoA^T A is interleaved
    between OT pairs 1 and 2 to cover the exp tail; softmax normalizer
    via a pairwise sum tree (first level on GpSimd, tail on DVE) +
    16.0-matrix matmul broadcast + fast reciprocal; OUT = OUT_A psum +
    OT*rb evicted by DVE adds to bf16.
  * PSUM: 7-deep ring for all matmul groups + 1 bank shared by warm-up
    and the normalizer broadcast.  The 7-deep ring gives the score
    stream enough slots that no psum is reused before its exp drains
    (exp is slower per tile than the DR matmuls that feed it).
  * All DRAM tensors are partition-major so every load/store is a single
    contiguous-burst DMA (2-4 KiB per partition line).  Output returns
    bf16 (host upcasts); the last block streams per-tile output DMAs on
    the fast-draining queues.  13 dummy matmuls on a memset tile warm
    the PE HAM clock-gate to 2.4 GHz while the first DMAs are in flight
    (sized to end right as ct/wq land, so the pstate ramp carries into
    the G prep).
"""

import numpy as np
import ml_dtypes

import concourse.tile as tile
from concourse import bacc, mybir
from concourse.bass_utils import run_bass_kernel_spmd

BF16 = mybir.dt.bfloat16
F32 = mybir.dt.float32
FP8 = mybir.dt.float8e4
NP_FP8 = mybir.dt.np(mybir.dt.float8e4)
DR = mybir.MatmulPerfMode.DoubleRow

C = 512      # in channels
E = 512      # emb dim
HW = 4096    # 64*64 image positions
L = 1024     # 32*32 context positions
P = 128      # partitions
B = 512      # positions per block
NBLK = HW // B    # 8
CT_T = C // P     # 4  tiles of input channels
ET = E // P       # 4  tiles of emb features
LT = L // P       # 8  tiles of context positions
SH = 1024.0  # scale folded into H (undone in the exp)
SV = 16.0    # scale folded into W_VO (undone in the normalizer)


def build_kernel():
    nc = bacc.Bacc("TRN2", target_bir_lowering=False, debug=False,
                   num_devices=8, enable_asserts=False)

    a_d = nc.dram_tensor("a", [NBLK, P, CT_T, B], BF16, kind="ExternalInput")
    a8_d = nc.dram_tensor("a8", [NBLK, P, CT_T, B], FP8, kind="ExternalInput")
    ct_d = nc.dram_tensor("ct", [P, ET, L], FP8, kind="ExternalInput")
    wq_d = nc.dram_tensor("wq", [P, ET, C], FP8, kind="ExternalInput")  # H'
    wv_d = nc.dram_tensor("wv", [P, ET, C], FP8, kind="ExternalInput")  # W_VO'
    wo_d = nc.dram_tensor("wo", [P, CT_T, C], BF16, kind="ExternalInput")
    out_d = nc.dram_tensor("out", [NBLK, P, CT_T, B], BF16,
                           kind="ExternalOutput")

    with tile.TileContext(nc) as tc:
        with (
            tc.tile_pool(name="const", bufs=1) as const,
            tc.tile_pool(name="ablk", bufs=3) as a_pool,
            tc.tile_pool(name="pt", bufs=2) as pt_pool,
            tc.tile_pool(name="otn", bufs=2) as otn_pool,
            tc.tile_pool(name="osb", bufs=2) as out_pool,
            tc.tile_pool(name="rb", bufs=2) as rb_pool,
            tc.tile_pool(name="tsum", bufs=2) as tsum_pool,
            tc.tile_pool(name="mmps", bufs=7, space="PSUM") as mm_psum,
            tc.tile_pool(name="smps", bufs=1, space="PSUM") as sm_psum,
        ):
            qs = [nc.sync, nc.gpsimd]

            # PE warm-up: dummy matmuls on a memset tile while the first
            # input DMAs are in flight, so the HAM clock-gate reaches
            # 8/8 (2.4 GHz) before the first real matmul issues.
            warm = const.tile([P, B], BF16)
            nc.vector.memset(warm, 1.0)
            wps = sm_psum.tile([P, B], F32, tag="small")
            for _ in range(13):
                nc.tensor.matmul(wps, warm[:, 0:P], warm, start=True,
                                 stop=True)
            warm_guard = const.tile([1, 1], F32)
            nc.vector.tensor_copy(out=warm_guard, in_=wps[0:1, 0:1])

            # startup-critical loads first: G needs ct + H'(wq)
            ct_sb = const.tile([P, ET, L], FP8)
            nc.sync.dma_start(out=ct_sb[:, 0:2], in_=ct_d.ap()[:, 0:2])
            nc.scalar.dma_start(out=ct_sb[:, 2:4], in_=ct_d.ap()[:, 2:4])
            wq_sb = const.tile([P, ET, C], FP8)
            nc.gpsimd.dma_start(out=wq_sb, in_=wq_d.ap())

            def load_a(ib):
                blk8 = a_pool.tile([P, CT_T, B], FP8, tag="a8")
                qs[ib % 2].dma_start(out=blk8, in_=a8_d.ap()[ib])
                blk = a_pool.tile([P, CT_T, B], BF16, tag="a")
                qs[(ib + 1) % 2].dma_start(out=blk, in_=a_d.ap()[ib])
                return blk, blk8

            wv_sb = const.tile([P, ET, C], FP8)
            nc.sync.dma_start(out=wv_sb, in_=wv_d.ap())
            ab0 = load_a(0)
            wo_sb = const.tile([P, CT_T, C], BF16)
            nc.gpsimd.dma_start(out=wo_sb, in_=wo_d.ap())
            ones_mat = const.tile([P, P], BF16)
            nc.vector.memset(ones_mat, SV)

            # prep-eviction engines (must have a PSUM read port)
            def evict(i, out, in_):
                if i % 2 == 0:
                    nc.scalar.copy(out=out, in_=in_)
                else:
                    nc.vector.tensor_copy(out=out, in_=in_)

            # ---- G = H'^T ctx  [C, L]  (fused Q proj, fp8 DR) ----------
            # n2-outer so ST(0)'s first score tiles unblock after 4 evicts
            g_sb = const.tile([P, CT_T, L], FP8)
            for n2 in range(L // B):
                for m in range(CT_T):
                    ps = mm_psum.tile([P, B], F32, tag="mm")
                    for kp in range(ET // 2):
                        nc.tensor.matmul(
                            ps,
                            wq_sb[:, 2 * kp:2 * kp + 2, m * P:(m + 1) * P],
                            ct_sb[:, 2 * kp:2 * kp + 2, n2 * B:(n2 + 1) * B],
                            start=(kp == 0),
                            stop=(kp == ET // 2 - 1),
                            perf_mode=DR,
                        )
                    evict(m, g_sb[:, m, n2 * B:(n2 + 1) * B], ps)

            # ---- per block of B positions ------------------------------
            def st_exp(a8_blk):
                """ST = G^T A8 (fp8 DR), PT = exp(ST/SH) fp8, bf16 sums."""
                pt_blk = pt_pool.tile([P, LT, B], FP8, tag="pt")
                for mj in range(LT):
                    ps = mm_psum.tile([P, B], F32, tag="mm")
                    for kp in range(CT_T // 2):
                        nc.tensor.matmul(
                            ps,
                            g_sb[:, 2 * kp:2 * kp + 2, mj * P:(mj + 1) * P],
                            a8_blk[:, 2 * kp:2 * kp + 2, :],
                            start=(kp == 0),
                            stop=(kp == CT_T // 2 - 1),
                            perf_mode=DR,
                        )
                    nc.scalar.activation(
                        out=pt_blk[:, mj, :], in_=ps,
                        func=mybir.ActivationFunctionType.Exp,
                        scale=1.0 / SH,
                    )

                # partial column sums: big first level on GpSimd (DVE
                # relief), cheap tail levels on DVE (short t1 chain)
                t4 = tsum_pool.tile([P, 4, B], BF16, tag="t4")
                nc.gpsimd.tensor_add(t4, pt_blk[:, 0:4, :], pt_blk[:, 4:8, :])
                t2 = tsum_pool.tile([P, 2, B], BF16, tag="t2")
                nc.vector.tensor_add(t2, t4[:, 0:2, :], t4[:, 2:4, :])
                t1 = tsum_pool.tile([P, B], BF16, tag="t1")
                nc.vector.tensor_add(t1, t2[:, 0, :], t2[:, 1, :])
                return pt_blk, t1

            def ot_pairs(ot_ps, pt_blk, pairs):
                """OT accumulation, pair-major: each DR pair of context
                tiles feeds all 4 output psums before the next pair, so
                the PE never waits on more exp tiles than necessary."""
                for pair in pairs:
                    for md in range(CT_T):
                        nc.tensor.matmul(
                            ot_ps[md],
                            v_sb[:, 2 * pair:2 * pair + 2,
                                 md * P:(md + 1) * P],
                            pt_blk[:, 2 * pair:2 * pair + 2, :],
                            start=(pair == 0),
                            stop=(pair == LT // 2 - 1),
                            perf_mode=DR,
                        )

            def attn_norm(ot_ps, t1):
                """rb = 1/(SV * colsum): 16.0-matrix matmul broadcasts the
                cross-partition sum; fast reciprocal; normalize OT."""
                b_ps = sm_psum.tile([P, B], F32, tag="small")
                nc.tensor.matmul(b_ps, ones_mat, t1, start=True, stop=True)
                rb_sb = rb_pool.tile([P, B], F32, tag="rb")
                nc.vector.reciprocal_approx_fast(out=rb_sb, in_=b_ps)

                oo_blk = otn_pool.tile([P, CT_T, B], F32, tag="otn")
                for md in range(CT_T):
                    nc.vector.tensor_tensor(
                        out=oo_blk[:, md, :], in0=ot_ps[md], in1=rb_sb,
                        op=mybir.AluOpType.mult,
                    )
                return oo_blk

            def out_block(ib, a_blk, oo_blk):
                """OUT = WoA^T A (bf16) + OT*rb -> bf16 -> DMA.  The last
                block streams per-tile DMAs (on the fast-draining sync and
                scalar queues) so the tail drains sooner."""
                last = ib == NBLK - 1
                o_sb = out_pool.tile([P, CT_T, B], BF16, tag="osb")
                for mo in range(CT_T):
                    ps = mm_psum.tile([P, B], F32, tag="mm")
                    for kc in range(CT_T):
                        nc.tensor.matmul(
                            ps,
                            wo_sb[:, kc, mo * P:(mo + 1) * P],
                            a_blk[:, kc, :],
                            start=(kc == 0),
                            stop=(kc == CT_T - 1),
                        )
                    nc.vector.tensor_tensor(
                        out=o_sb[:, mo, :], in0=ps, in1=oo_blk[:, mo, :],
                        op=mybir.AluOpType.add,
                    )
                    if last:
                        (nc.sync if mo % 2 == 0 else nc.scalar).dma_start(
                            out=out_d.ap()[ib, :, mo], in_=o_sb[:, mo, :])
                if not last:
                    qs[ib % 2].dma_start(out=out_d.ap()[ib], in_=o_sb)

            # ---- software-pipelined main loop ---------------------------
            # Block 0's scores are issued BEFORE the VV prep so its exp
            # stream overlaps the VV matmuls instead of following them.
            a_blk, a8_blk = ab0
            pt_blk, t1 = st_exp(a8_blk)

            # VV = ctx^T W_VO'  [L, C]  (fused output proj, fp8 DR)
            v_sb = const.tile([P, LT, C], FP8)
            for mj in range(LT):
                ps = mm_psum.tile([P, C], F32, tag="mm")
                for kp in range(ET // 2):
                    nc.tensor.matmul(
                        ps,
                        ct_sb[:, 2 * kp:2 * kp + 2, mj * P:(mj + 1) * P],
                        wv_sb[:, 2 * kp:2 * kp + 2, :],
                        start=(kp == 0),
                        stop=(kp == ET // 2 - 1),
                        perf_mode=DR,
                    )
                evict(mj + 1, v_sb[:, mj, :], ps)

            prev = None  # (ib, a_blk, oo_blk)
            for ib in range(NBLK):
                ot_ps = [mm_psum.tile([P, B], F32, tag="mm",
                                      name=f"ot_ps{ib}_{md}")
                         for md in range(CT_T)]
                ot_pairs(ot_ps, pt_blk, (0, 1))
                nxt = load_a(ib + 1) if ib + 1 < NBLK else None
                if prev is not None:
                    out_block(*prev)
                ot_pairs(ot_ps, pt_blk, (2, 3))
                oo_blk = attn_norm(ot_ps, t1)
                prev = (ib, a_blk, oo_blk)
                if nxt is not None:
                    a_blk, a8_blk = nxt
                    pt_blk, t1 = st_exp(a8_blk)
            out_block(*prev)

    nc.compile()
    return nc


_NC = None


def _get_nc():
    global _NC
    if _NC is None:
        _NC = build_kernel()
    return _NC


def run(inputs: dict, trace: bool = False):
    """Shard inputs over 8 cores, run the SPMD kernel, gather the output."""
    bf = ml_dtypes.bfloat16
    inp = np.asarray(inputs["input"], np.float32).reshape(8, C, HW)
    ctx = np.asarray(inputs["context"], np.float32).reshape(8, E, L)
    proj_in_w = np.asarray(inputs["proj_in_w"], np.float32)
    wq_w = np.asarray(inputs["wq_w"], np.float32)
    wk_w = np.asarray(inputs["wk_w"], np.float32)
    wv_w = np.asarray(inputs["wv_w"], np.float32)
    proj_out_w = np.asarray(inputs["proj_out_w"], np.float32)

    def fp8(x):
        return np.clip(x, -240.0, 240.0).astype(NP_FP8)

    scale = float(E) ** -0.5
    wq_eff = (proj_in_w.T @ wq_w) * scale         # [C, E]
    h_w = SH * (wk_w @ wq_eff.T)                  # [E, C]
    wo_full = proj_out_w.T                        # [C+E, C]
    w_vo = SV * (wv_w @ wo_full[C:])              # [E, C]
    wo_a = wo_full[:C]                            # [C, C]

    # partition-major retiles: one contiguous-burst DMA per tensor/block
    h8 = fp8(h_w.reshape(ET, P, C).transpose(1, 0, 2))
    wv8 = fp8(w_vo.reshape(ET, P, C).transpose(1, 0, 2))
    wo16 = np.ascontiguousarray(
        wo_a.reshape(CT_T, P, C).transpose(1, 0, 2)).astype(bf)
    ct8 = fp8(ctx.reshape(8, ET, P, L).transpose(0, 2, 1, 3))
    a_t = inp.reshape(8, CT_T, P, NBLK, B).transpose(0, 3, 2, 1, 4)
    a16 = np.ascontiguousarray(a_t).astype(bf)    # [8, blk, p, ct, b]
    a8 = fp8(a_t)

    in_maps = [
        {
            "a": a16[i],
            "a8": a8[i],
            "ct": ct8[i],
            "wq": h8,
            "wv": wv8,
            "wo": wo16,
        }
        for i in range(8)
    ]

    nc = _get_nc()
    res = run_bass_kernel_spmd(nc, in_maps, core_ids=list(range(8)),
                               trace=trace)
    out = np.stack([res.results[i]["out"] for i in range(8)])
    # [8, blk, p, ct, b] -> [8, C, 64, 64]
    out = out.astype(np.float32).transpose(0, 3, 2, 1, 4).reshape(8, C, 64, 64)
    return np.ascontiguousarray(out), res


def kernel(**inputs) -> np.ndarray:
    out, _ = run(inputs, trace=False)
    return out


# revision 22
# speedup vs baseline: 1.0081x; 1.0081x over previous
"""Trainium2 Bass kernel for nn_CrossAttention (batch-parallel over 8 cores).

Reference computation (per batch element b):
    x   = proj_in(input)              # 1x1 conv -> [hw, emb]
    Q   = x @ wq ;  K = ctx @ wk ; V = ctx @ wv
    att = softmax(Q K^T * emb^-0.5)
    out = att @ V                     # [hw, emb]
    out = proj_out(concat([input, out], ch))   # 1x1 conv -> [in_ch, h, w]

Device strategy (data-parallel, one batch element per NeuronCore):
  * Host folds proj_in into the Q projection and fuses it into the scores:
        att^T = G^T A,  G = H^T ctx,  H = wk Wq_eff^T,
        Wq_eff = proj_in_w.T wq * emb^-0.5
    wv and the attention half of proj_out fold into one matrix W_VO, so
        out = WoA^T A + (ctx^T W_VO)^T softmax-weights.
  * The attention path (G, VV, scores, att @ V) runs in fp8e4m3 with
    DoubleRow matmuls (two 128-row k-tiles per instruction, 2x PE
    throughput).  Its contribution to the output is ~75x smaller than
    the direct WoA^T A path, so fp8 error there is negligible; the
    direct path stays bf16.  Power-of-two scales keep fp8 operands in
    the normal range: H is scaled by 1024 (undone by exp(x/1024) via the
    activation scale), W_VO by 16 (undone by a 16.0 "ones" matrix in the
    softmax-normalizer matmul, so rb = 1/(16*colsum)).
  * Per 512-position block: ST = G^T A8 (8 psum groups, 2 DR matmuls
    each) -> exp -> PT fp8; OT = VV^T PT issued PAIR-major (md inner) so
    each OT pair consumes exp tiles right as the scalar engine finishes
    them; the previous block's bf16 OUT_A = WoA^T A is interleaved
    between OT pairs 1 and 2 to cover the exp tail; softmax normalizer
    via a pairwise sum tree (first level on GpSimd, tail on DVE) +
    16.0-matrix matmul broadcast + fast reciprocal; OUT = OUT_A psum +
    OT*rb evicted by DVE adds to bf16.
  * PSUM: 7-deep ring for all matmul groups + 1 bank shared by warm-up
    and the normalizer broadcast.  The 7-deep ring gives the score
    stream enough slots that no psum is reused before its exp drains
    (exp is slower per tile than the DR matmuls that feed it).
  * All DRAM tensors are partition-major so every load/store is a single
    contiguous-burst DMA (2-4 KiB per partition line).  Output returns
    bf16 (host upcasts); the last block streams per-tile output DMAs on
    the fast-draining queues.  13 dummy matmuls on a memset tile warm
    the PE HAM clock-gate to 2.4 GHz while the first DMAs are in flight
    (sized to end right as ct/wq land, so the pstate ramp carries into
    the G prep).
"""

import numpy as np
import ml_dtypes

import concourse.tile as tile
from concourse import bacc, mybir
from concourse.bass_utils import run_bass_kernel_spmd

BF16 = mybir.dt.bfloat16
F32 = mybir.dt.float32
FP8 = mybir.dt.float8e4
NP_FP8 = mybir.dt.np(mybir.dt.float8e4)
DR = mybir.MatmulPerfMode.DoubleRow

C = 512      # in channels
E = 512      # emb dim
HW = 4096    # 64*64 image positions
L = 1024     # 32*32 context positions
P = 128      # partitions
B = 512      # positions per block
NBLK = HW // B    # 8
CT_T = C // P     # 4  tiles of input channels
ET = E // P       # 4  tiles of emb features
LT = L // P       # 8  tiles of context positions
SH = 1024.0  # scale folded into H (undone in the exp)
SV = 16.0    # scale folded into W_VO (undone in the normalizer)


def build_kernel():
    nc = bacc.Bacc("TRN2", target_bir_lowering=False, debug=False,
                   num_devices=8, enable_asserts=False)

    a_d = nc.dram_tensor("a", [NBLK, P, CT_T, B], BF16, kind="ExternalInput")
    a8_d = nc.dram_tensor("a8", [NBLK, P, CT_T, B], FP8, kind="ExternalInput")
    ct_d = nc.dram_tensor("ct", [P, ET, L], FP8, kind="ExternalInput")
    wq_d = nc.dram_tensor("wq", [P, ET, C], FP8, kind="ExternalInput")  # H'
    wv_d = nc.dram_tensor("wv", [P, ET, C], FP8, kind="ExternalInput")  # W_VO'
    wo_d = nc.dram_tensor("wo", [P, CT_T, C], BF16, kind="ExternalInput")
    out_d = nc.dram_tensor("out", [NBLK, P, CT_T, B], BF16,
                           kind="ExternalOutput")

    with tile.TileContext(nc) as tc:
        with (
            tc.tile_pool(name="const", bufs=1) as const,
            tc.tile_pool(name="ablk", bufs=3) as a_pool,
            tc.tile_pool(name="pt", bufs=2) as pt_pool,
            tc.tile_pool(name="otn", bufs=2) as otn_pool,
            tc.tile_pool(name="osb", bufs=2) as out_pool,
            tc.tile_pool(name="rb", bufs=2) as rb_pool,
            tc.tile_pool(name="tsum", bufs=2) as tsum_pool,
            tc.tile_pool(name="mmps", bufs=7, space="PSUM") as mm_psum,
            tc.tile_pool(name="smps", bufs=1, space="PSUM") as sm_psum,
        ):
            qs = [nc.sync, nc.gpsimd]

            # PE warm-up: dummy matmuls on a memset tile while the first
            # input DMAs are in flight, so the HAM clock-gate reaches
            # 8/8 (2.4 GHz) before the first real matmul issues.
            warm = const.tile([P, B], BF16)
            nc.vector.memset(warm, 1.0)
            wps = sm_psum.tile([P, B], F32, tag="small")
            for _ in range(14):
                nc.tensor.matmul(wps, warm[:, 0:P], warm, start=True,
                                 stop=True)
            warm_guard = const.tile([1, 1], F32)
            nc.vector.tensor_copy(out=warm_guard, in_=wps[0:1, 0:1])

            # startup-critical loads first: G needs ct + H'(wq)
            # ct split by L-halves: G's first half (n2=0) and the first
            # score/VV column tiles depend only on the first DMA
            ct_sb = const.tile([P, ET, L], FP8)
            nc.sync.dma_start(out=ct_sb[:, :, 0:B], in_=ct_d.ap()[:, :, 0:B])
            nc.scalar.dma_start(out=ct_sb[:, :, B:L], in_=ct_d.ap()[:, :, B:L])
            wq_sb = const.tile([P, ET, C], FP8)
            nc.gpsimd.dma_start(out=wq_sb, in_=wq_d.ap())

            def load_a(ib):
                blk8 = a_pool.tile([P, CT_T, B], FP8, tag="a8")
                qs[ib % 2].dma_start(out=blk8, in_=a8_d.ap()[ib])
                blk = a_pool.tile([P, CT_T, B], BF16, tag="a")
                qs[(ib + 1) % 2].dma_start(out=blk, in_=a_d.ap()[ib])
                return blk, blk8

            wv_sb = const.tile([P, ET, C], FP8)
            nc.sync.dma_start(out=wv_sb, in_=wv_d.ap())
            ab0 = load_a(0)
            wo_sb = const.tile([P, CT_T, C], BF16)
            nc.gpsimd.dma_start(out=wo_sb, in_=wo_d.ap())
            ones_mat = const.tile([P, P], BF16)
            nc.vector.memset(ones_mat, SV)

            # prep-eviction engines (must have a PSUM read port)
            def evict(i, out, in_):
                if i % 2 == 0:
                    nc.scalar.copy(out=out, in_=in_)
                else:
                    nc.vector.tensor_copy(out=out, in_=in_)

            # ---- G = H'^T ctx  [C, L]  (fused Q proj, fp8 DR) ----------
            # n2-outer so ST(0)'s first score tiles unblock after 4 evicts
            g_sb = const.tile([P, CT_T, L], FP8)
            for n2 in range(L // B):
                for m in range(CT_T):
                    ps = mm_psum.tile([P, B], F32, tag="mm")
                    for kp in range(ET // 2):
                        nc.tensor.matmul(
                            ps,
                            wq_sb[:, 2 * kp:2 * kp + 2, m * P:(m + 1) * P],
                            ct_sb[:, 2 * kp:2 * kp + 2, n2 * B:(n2 + 1) * B],
                            start=(kp == 0),
                            stop=(kp == ET // 2 - 1),
                            perf_mode=DR,
                        )
                    evict(m, g_sb[:, m, n2 * B:(n2 + 1) * B], ps)

            # ---- per block of B positions ------------------------------
            def st_exp(a8_blk):
                """ST = G^T A8 (fp8 DR), PT = exp(ST/SH) fp8, bf16 sums."""
                pt_blk = pt_pool.tile([P, LT, B], FP8, tag="pt")
                for mj in range(LT):
                    ps = mm_psum.tile([P, B], F32, tag="mm")
                    for kp in range(CT_T // 2):
                        nc.tensor.matmul(
                            ps,
                            g_sb[:, 2 * kp:2 * kp + 2, mj * P:(mj + 1) * P],
                            a8_blk[:, 2 * kp:2 * kp + 2, :],
                            start=(kp == 0),
                            stop=(kp == CT_T // 2 - 1),
                            perf_mode=DR,
                        )
                    nc.scalar.activation(
                        out=pt_blk[:, mj, :], in_=ps,
                        func=mybir.ActivationFunctionType.Exp,
                        scale=1.0 / SH,
                    )

                # partial column sums: big first level on GpSimd (DVE
                # relief), cheap tail levels on DVE (short t1 chain)
                t4 = tsum_pool.tile([P, 4, B], BF16, tag="t4")
                nc.gpsimd.tensor_add(t4, pt_blk[:, 0:4, :], pt_blk[:, 4:8, :])
                t2 = tsum_pool.tile([P, 2, B], BF16, tag="t2")
                nc.vector.tensor_add(t2, t4[:, 0:2, :], t4[:, 2:4, :])
                t1 = tsum_pool.tile([P, B], BF16, tag="t1")
                nc.vector.tensor_add(t1, t2[:, 0, :], t2[:, 1, :])
                return pt_blk, t1

            def ot_pairs(ot_ps, pt_blk, pairs):
                """OT accumulation, pair-major: each DR pair of context
                tiles feeds all 4 output psums before the next pair, so
                the PE never waits on more exp tiles than necessary."""
                for pair in pairs:
                    for md in range(CT_T):
                        nc.tensor.matmul(
                            ot_ps[md],
                            v_sb[:, 2 * pair:2 * pair + 2,
                                 md * P:(md + 1) * P],
                            pt_blk[:, 2 * pair:2 * pair + 2, :],
                            start=(pair == 0),
                            stop=(pair == LT // 2 - 1),
                            perf_mode=DR,
                        )

            def attn_norm(ot_ps, t1):
                """rb = 1/(SV * colsum): 16.0-matrix matmul broadcasts the
                cross-partition sum; fast reciprocal; normalize OT."""
                b_ps = sm_psum.tile([P, B], F32, tag="small")
                nc.tensor.matmul(b_ps, ones_mat, t1, start=True, stop=True)
                rb_sb = rb_pool.tile([P, B], F32, tag="rb")
                nc.vector.reciprocal_approx_fast(out=rb_sb, in_=b_ps)

                oo_blk = otn_pool.tile([P, CT_T, B], F32, tag="otn")
                for md in range(CT_T):
                    nc.vector.tensor_tensor(
                        out=oo_blk[:, md, :], in0=ot_ps[md], in1=rb_sb,
                        op=mybir.AluOpType.mult,
                    )
                return oo_blk

            def out_block(ib, a_blk, oo_blk):
                """OUT = WoA^T A (bf16) + OT*rb -> bf16 -> DMA.  The last
                block streams per-tile DMAs (on the fast-draining sync and
                scalar queues) so the tail drains sooner."""
                last = ib == NBLK - 1
                o_sb = out_pool.tile([P, CT_T, B], BF16, tag="osb")
                for mo in range(CT_T):
                    ps = mm_psum.tile([P, B], F32, tag="mm")
                    for kc in range(CT_T):
                        nc.tensor.matmul(
                            ps,
                            wo_sb[:, kc, mo * P:(mo + 1) * P],
                            a_blk[:, kc, :],
                            start=(kc == 0),
                            stop=(kc == CT_T - 1),
                        )
                    nc.vector.tensor_tensor(
                        out=o_sb[:, mo, :], in0=ps, in1=oo_blk[:, mo, :],
                        op=mybir.AluOpType.add,
                    )
                    if last:
                        (nc.sync if mo % 2 == 0 else nc.scalar).dma_start(
                            out=out_d.ap()[ib, :, mo], in_=o_sb[:, mo, :])
                if not last:
                    qs[ib % 2].dma_start(out=out_d.ap()[ib], in_=o_sb)

            # ---- software-pipelined main loop ---------------------------
            # Block 0's scores are issued BEFORE the VV prep so its exp
            # stream overlaps the VV matmuls instead of following them.
            a_blk, a8_blk = ab0
            pt_blk, t1 = st_exp(a8_blk)

            # VV = ctx^T W_VO'  [L, C]  (fused output proj, fp8 DR)
            v_sb = const.tile([P, LT, C], FP8)
            for mj in range(LT):
                ps = mm_psum.tile([P, C], F32, tag="mm")
                for kp in range(ET // 2):
                    nc.tensor.matmul(
                        ps,
                        ct_sb[:, 2 * kp:2 * kp + 2, mj * P:(mj + 1) * P],
                        wv_sb[:, 2 * kp:2 * kp + 2, :],
                        start=(kp == 0),
                        stop=(kp == ET // 2 - 1),
                        perf_mode=DR,
                    )
                evict(mj + 1, v_sb[:, mj, :], ps)

            prev = None  # (ib, a_blk, oo_blk)
            for ib in range(NBLK):
                ot_ps = [mm_psum.tile([P, B], F32, tag="mm",
                                      name=f"ot_ps{ib}_{md}")
                         for md in range(CT_T)]
                ot_pairs(ot_ps, pt_blk, (0, 1))
                nxt = load_a(ib + 1) if ib + 1 < NBLK else None
                if prev is not None:
                    out_block(*prev)
                ot_pairs(ot_ps, pt_blk, (2, 3))
                oo_blk = attn_norm(ot_ps, t1)
                prev = (ib, a_blk, oo_blk)
                if nxt is not None:
                    a_blk, a8_blk = nxt
                    pt_blk, t1 = st_exp(a8_blk)
            out_block(*prev)

    nc.compile()
    return nc


_NC = None


def _get_nc():
    global _NC
    if _NC is None:
        _NC = build_kernel()
    return _NC


def run(inputs: dict, trace: bool = False):
    """Shard inputs over 8 cores, run the SPMD kernel, gather the output."""
    bf = ml_dtypes.bfloat16
    inp = np.asarray(inputs["input"], np.float32).reshape(8, C, HW)
    ctx = np.asarray(inputs["context"], np.float32).reshape(8, E, L)
    proj_in_w = np.asarray(inputs["proj_in_w"], np.float32)
    wq_w = np.asarray(inputs["wq_w"], np.float32)
    wk_w = np.asarray(inputs["wk_w"], np.float32)
    wv_w = np.asarray(inputs["wv_w"], np.float32)
    proj_out_w = np.asarray(inputs["proj_out_w"], np.float32)

    def fp8(x):
        return np.clip(x, -240.0, 240.0).astype(NP_FP8)

    scale = float(E) ** -0.5
    wq_eff = (proj_in_w.T @ wq_w) * scale         # [C, E]
    h_w = SH * (wk_w @ wq_eff.T)                  # [E, C]
    wo_full = proj_out_w.T                        # [C+E, C]
    w_vo = SV * (wv_w @ wo_full[C:])              # [E, C]
    wo_a = wo_full[:C]                            # [C, C]

    # partition-major retiles: one contiguous-burst DMA per tensor/block
    h8 = fp8(h_w.reshape(ET, P, C).transpose(1, 0, 2))
    wv8 = fp8(w_vo.reshape(ET, P, C).transpose(1, 0, 2))
    wo16 = np.ascontiguousarray(
        wo_a.reshape(CT_T, P, C).transpose(1, 0, 2)).astype(bf)
    ct8 = fp8(ctx.reshape(8, ET, P, L).transpose(0, 2, 1, 3))
    a_t = inp.reshape(8, CT_T, P, NBLK, B).transpose(0, 3, 2, 1, 4)
    a16 = np.ascontiguousarray(a_t).astype(bf)    # [8, blk, p, ct, b]
    a8 = fp8(a_t)

    in_maps = [
        {
            "a": a16[i],
            "a8": a8[i],
            "ct": ct8[i],
            "wq": h8,
            "wv": wv8,
            "wo": wo16,
        }
        for i in range(8)
    ]

    nc = _get_nc()
    res = run_bass_kernel_spmd(nc, in_maps, core_ids=list(range(8)),
                               trace=trace)
    out = np.stack([res.results[i]["out"] for i in range(8)])
    # [8, blk, p, ct, b] -> [8, C, 64, 64]
    out = out.astype(np.float32).transpose(0, 3, 2, 1, 4).reshape(8, C, 64, 64)
    return np.ascontiguousarray(out), res


def kernel(**inputs) -> np.ndarray:
    out, _ = run(inputs, trace=False)
    return out


# revision 24
# speedup vs baseline: 1.0142x; 1.0061x over previous
"""Trainium2 Bass kernel for nn_CrossAttention (batch-parallel over 8 cores).

Reference computation (per batch element b):
    x   = proj_in(input)              # 1x1 conv -> [hw, emb]
    Q   = x @ wq ;  K = ctx @ wk ; V = ctx @ wv
    att = softmax(Q K^T * emb^-0.5)
    out = att @ V                     # [hw, emb]
    out = proj_out(concat([input, out], ch))   # 1x1 conv -> [in_ch, h, w]

Device strategy (data-parallel, one batch element per NeuronCore):
  * Host folds proj_in into the Q projection and fuses it into the scores:
        att^T = G^T A,  G = H^T ctx,  H = wk Wq_eff^T,
        Wq_eff = proj_in_w.T wq * emb^-0.5
    wv and the attention half of proj_out fold into one matrix W_VO, so
        out = WoA^T A + (ctx^T W_VO)^T softmax-weights.
  * The attention path (G, VV, scores, att @ V) runs in fp8e4m3 with
    DoubleRow matmuls (two 128-row k-tiles per instruction, 2x PE
    throughput).  Its contribution to the output is ~75x smaller than
    the direct WoA^T A path, so fp8 error there is negligible; the
    direct path stays bf16.  Power-of-two scales keep fp8 operands in
    the normal range: H is scaled by 1024 (undone by exp(x/1024) via the
    activation scale), W_VO by 16 (undone by a 16.0 "ones" matrix in the
    softmax-normalizer matmul, so rb = 1/(16*colsum)).
  * Per 512-position block: ST = G^T A8 (8 psum groups, 2 DR matmuls
    each) -> exp -> PT fp8; OT = VV^T PT issued PAIR-major (md inner) so
    each OT pair consumes exp tiles right as the scalar engine finishes
    them; the previous block's bf16 OUT_A = WoA^T A is interleaved
    between OT pairs 1 and 2 to cover the exp tail; softmax normalizer
    via a pairwise sum tree (first level on GpSimd, tail on DVE) +
    16.0-matrix matmul broadcast + fast reciprocal; OUT = OUT_A psum +
    OT*rb evicted by DVE adds to bf16.
  * PSUM: 7-deep ring for all matmul groups + 1 bank shared by warm-up
    and the normalizer broadcast.  The 7-deep ring gives the score
    stream enough slots that no psum is reused before its exp drains
    (exp is slower per tile than the DR matmuls that feed it).
  * All DRAM tensors are partition-major so every load/store is a single
    contiguous-burst DMA (2-4 KiB per partition line).  Output returns
    bf16 (host upcasts); the last block streams per-tile output DMAs on
    the fast-draining queues.  13 dummy matmuls on a memset tile warm
    the PE HAM clock-gate to 2.4 GHz while the first DMAs are in flight
    (sized to end right as ct/wq land, so the pstate ramp carries into
    the G prep).
"""

import numpy as np
import ml_dtypes

import concourse.tile as tile
from concourse import bacc, mybir
from concourse.bass_utils import run_bass_kernel_spmd

BF16 = mybir.dt.bfloat16
F32 = mybir.dt.float32
FP8 = mybir.dt.float8e4
NP_FP8 = mybir.dt.np(mybir.dt.float8e4)
DR = mybir.MatmulPerfMode.DoubleRow

C = 512      # in channels
E = 512      # emb dim
HW = 4096    # 64*64 image positions
L = 1024     # 32*32 context positions
P = 128      # partitions
B = 512      # positions per block
NBLK = HW // B    # 8
CT_T = C // P     # 4  tiles of input channels
ET = E // P       # 4  tiles of emb features
LT = L // P       # 8  tiles of context positions
SH = 1024.0  # scale folded into H (undone in the exp)
SV = 16.0    # scale folded into W_VO (undone in the normalizer)


def build_kernel():
    nc = bacc.Bacc("TRN2", target_bir_lowering=False, debug=False,
                   num_devices=8, enable_asserts=False)

    a_d = nc.dram_tensor("a", [NBLK, P, CT_T, B], BF16, kind="ExternalInput")
    a8_d = nc.dram_tensor("a8", [NBLK, P, CT_T, B], FP8, kind="ExternalInput")
    ct_d = nc.dram_tensor("ct", [P, ET, L], FP8, kind="ExternalInput")
    wq_d = nc.dram_tensor("wq", [P, ET, C], FP8, kind="ExternalInput")  # H'
    wv_d = nc.dram_tensor("wv", [P, ET, C], FP8, kind="ExternalInput")  # W_VO'
    wo_d = nc.dram_tensor("wo", [P, CT_T, C], BF16, kind="ExternalInput")
    out_d = nc.dram_tensor("out", [NBLK, P, CT_T, B], BF16,
                           kind="ExternalOutput")

    with tile.TileContext(nc) as tc:
        with (
            tc.tile_pool(name="const", bufs=1) as const,
            tc.tile_pool(name="ablk", bufs=3) as a_pool,
            tc.tile_pool(name="pt", bufs=2) as pt_pool,
            tc.tile_pool(name="otn", bufs=2) as otn_pool,
            tc.tile_pool(name="osb", bufs=2) as out_pool,
            tc.tile_pool(name="rb", bufs=2) as rb_pool,
            tc.tile_pool(name="tsum", bufs=2) as tsum_pool,
            tc.tile_pool(name="mmps", bufs=7, space="PSUM") as mm_psum,
            tc.tile_pool(name="smps", bufs=1, space="PSUM") as sm_psum,
        ):
            qs = [nc.sync, nc.gpsimd]

            # PE warm-up: dummy matmuls on a memset tile while the first
            # input DMAs are in flight, so the HAM clock-gate reaches
            # 8/8 (2.4 GHz) before the first real matmul issues.
            warm = const.tile([P, B], BF16)
            nc.vector.memset(warm, 1.0)
            wps = sm_psum.tile([P, B], F32, tag="small")
            for _ in range(14):
                nc.tensor.matmul(wps, warm[:, 0:P], warm, start=True,
                                 stop=True)
            # guard on scalar: keeps the warm matmuls live without
            # blocking vector's early prep evictions behind the warm psum
            warm_guard = const.tile([1, 1], F32)
            nc.scalar.copy(out=warm_guard, in_=wps[0:1, 0:1])

            # startup-critical loads first: G needs ct + H'(wq)
            # ct split by L-halves: G's first half (n2=0) and the first
            # score/VV column tiles depend only on the first DMA
            ct_sb = const.tile([P, ET, L], FP8)
            nc.sync.dma_start(out=ct_sb[:, :, 0:B], in_=ct_d.ap()[:, :, 0:B])
            nc.scalar.dma_start(out=ct_sb[:, :, B:L], in_=ct_d.ap()[:, :, B:L])
            wq_sb = const.tile([P, ET, C], FP8)
            nc.gpsimd.dma_start(out=wq_sb, in_=wq_d.ap())

            def load_a(ib):
                blk8 = a_pool.tile([P, CT_T, B], FP8, tag="a8")
                qs[ib % 2].dma_start(out=blk8, in_=a8_d.ap()[ib])
                blk = a_pool.tile([P, CT_T, B], BF16, tag="a")
                qs[(ib + 1) % 2].dma_start(out=blk, in_=a_d.ap()[ib])
                return blk, blk8

            wv_sb = const.tile([P, ET, C], FP8)
            nc.sync.dma_start(out=wv_sb, in_=wv_d.ap())
            ab0 = load_a(0)
            wo_sb = const.tile([P, CT_T, C], BF16)
            nc.gpsimd.dma_start(out=wo_sb, in_=wo_d.ap())
            ones_mat = const.tile([P, P], BF16)
            nc.vector.memset(ones_mat, SV)

            # prep-eviction engines (must have a PSUM read port)
            def evict(i, out, in_):
                if i % 2 == 0:
                    nc.scalar.copy(out=out, in_=in_)
                else:
                    nc.vector.tensor_copy(out=out, in_=in_)

            # ---- G = H'^T ctx  [C, L]  (fused Q proj, fp8 DR) ----------
            # n2-outer so ST(0)'s first score tiles unblock after 4 evicts
            g_sb = const.tile([P, CT_T, L], FP8)
            for n2 in range(L // B):
                for m in range(CT_T):
                    ps = mm_psum.tile([P, B], F32, tag="mm")
                    for kp in range(ET // 2):
                        nc.tensor.matmul(
                            ps,
                            wq_sb[:, 2 * kp:2 * kp + 2, m * P:(m + 1) * P],
                            ct_sb[:, 2 * kp:2 * kp + 2, n2 * B:(n2 + 1) * B],
                            start=(kp == 0),
                            stop=(kp == ET // 2 - 1),
                            perf_mode=DR,
                        )
                    evict(m + 1, g_sb[:, m, n2 * B:(n2 + 1) * B], ps)

            # ---- per block of B positions ------------------------------
            def st_exp(a8_blk):
                """ST = G^T A8 (fp8 DR), PT = exp(ST/SH) fp8, bf16 sums."""
                pt_blk = pt_pool.tile([P, LT, B], FP8, tag="pt")
                for mj in range(LT):
                    ps = mm_psum.tile([P, B], F32, tag="mm")
                    for kp in range(CT_T // 2):
                        nc.tensor.matmul(
                            ps,
                            g_sb[:, 2 * kp:2 * kp + 2, mj * P:(mj + 1) * P],
                            a8_blk[:, 2 * kp:2 * kp + 2, :],
                            start=(kp == 0),
                            stop=(kp == CT_T // 2 - 1),
                            perf_mode=DR,
                        )
                    nc.scalar.activation(
                        out=pt_blk[:, mj, :], in_=ps,
                        func=mybir.ActivationFunctionType.Exp,
                        scale=1.0 / SH,
                    )

                # partial column sums: big first level on GpSimd (DVE
                # relief), cheap tail levels on DVE (short t1 chain)
                t4 = tsum_pool.tile([P, 4, B], BF16, tag="t4")
                nc.gpsimd.tensor_add(t4, pt_blk[:, 0:4, :], pt_blk[:, 4:8, :])
                t2 = tsum_pool.tile([P, 2, B], BF16, tag="t2")
                nc.vector.tensor_add(t2, t4[:, 0:2, :], t4[:, 2:4, :])
                t1 = tsum_pool.tile([P, B], BF16, tag="t1")
                nc.vector.tensor_add(t1, t2[:, 0, :], t2[:, 1, :])
                return pt_blk, t1

            def ot_pairs(ot_ps, pt_blk, pairs):
                """OT accumulation, pair-major: each DR pair of context
                tiles feeds all 4 output psums before the next pair, so
                the PE never waits on more exp tiles than necessary."""
                for pair in pairs:
                    for md in range(CT_T):
                        nc.tensor.matmul(
                            ot_ps[md],
                            v_sb[:, 2 * pair:2 * pair + 2,
                                 md * P:(md + 1) * P],
                            pt_blk[:, 2 * pair:2 * pair + 2, :],
                            start=(pair == 0),
                            stop=(pair == LT // 2 - 1),
                            perf_mode=DR,
                        )

            def attn_norm(ot_ps, t1):
                """rb = 1/(SV * colsum): 16.0-matrix matmul broadcasts the
                cross-partition sum; fast reciprocal; normalize OT."""
                b_ps = sm_psum.tile([P, B], F32, tag="small")
                nc.tensor.matmul(b_ps, ones_mat, t1, start=True, stop=True)
                rb_sb = rb_pool.tile([P, B], F32, tag="rb")
                nc.vector.reciprocal_approx_fast(out=rb_sb, in_=b_ps)

                oo_blk = otn_pool.tile([P, CT_T, B], F32, tag="otn")
                for md in range(CT_T):
                    nc.vector.tensor_tensor(
                        out=oo_blk[:, md, :], in0=ot_ps[md], in1=rb_sb,
                        op=mybir.AluOpType.mult,
                    )
                return oo_blk

            def out_block(ib, a_blk, oo_blk):
                """OUT = WoA^T A (bf16) + OT*rb -> bf16 -> DMA.  The last
                block streams per-tile DMAs (on the fast-draining sync and
                scalar queues) so the tail drains sooner."""
                last = ib == NBLK - 1
                o_sb = out_pool.tile([P, CT_T, B], BF16, tag="osb")
                for mo in range(CT_T):
                    ps = mm_psum.tile([P, B], F32, tag="mm")
                    for kc in range(CT_T):
                        nc.tensor.matmul(
                            ps,
                            wo_sb[:, kc, mo * P:(mo + 1) * P],
                            a_blk[:, kc, :],
                            start=(kc == 0),
                            stop=(kc == CT_T - 1),
                        )
                    nc.vector.tensor_tensor(
                        out=o_sb[:, mo, :], in0=ps, in1=oo_blk[:, mo, :],
                        op=mybir.AluOpType.add,
                    )
                    if last:
                        (nc.sync if mo % 2 == 0 else nc.scalar).dma_start(
                            out=out_d.ap()[ib, :, mo], in_=o_sb[:, mo, :])
                if not last:
                    qs[ib % 2].dma_start(out=out_d.ap()[ib], in_=o_sb)

            # ---- software-pipelined main loop ---------------------------
            # Block 0's scores are issued BEFORE the VV prep so its exp
            # stream overlaps the VV matmuls instead of following them.
            a_blk, a8_blk = ab0
            pt_blk, t1 = st_exp(a8_blk)

            # VV = ctx^T W_VO'  [L, C]  (fused output proj, fp8 DR)
            v_sb = const.tile([P, LT, C], FP8)
            for mj in range(LT):
                ps = mm_psum.tile([P, C], F32, tag="mm")
                for kp in range(ET // 2):
                    nc.tensor.matmul(
                        ps,
                        ct_sb[:, 2 * kp:2 * kp + 2, mj * P:(mj + 1) * P],
                        wv_sb[:, 2 * kp:2 * kp + 2, :],
                        start=(kp == 0),
                        stop=(kp == ET // 2 - 1),
                        perf_mode=DR,
                    )
                evict(mj + 1, v_sb[:, mj, :], ps)

            prev = None  # (ib, a_blk, oo_blk)
            for ib in range(NBLK):
                ot_ps = [mm_psum.tile([P, B], F32, tag="mm",
                                      name=f"ot_ps{ib}_{md}")
                         for md in range(CT_T)]
                ot_pairs(ot_ps, pt_blk, (0, 1))
                nxt = load_a(ib + 1) if ib + 1 < NBLK else None
                if prev is not None:
                    out_block(*prev)
                ot_pairs(ot_ps, pt_blk, (2, 3))
                oo_blk = attn_norm(ot_ps, t1)
                prev = (ib, a_blk, oo_blk)
                if nxt is not None:
                    a_blk, a8_blk = nxt
                    pt_blk, t1 = st_exp(a8_blk)
            out_block(*prev)

    nc.compile()
    return nc


_NC = None


def _get_nc():
    global _NC
    if _NC is None:
        _NC = build_kernel()
    return _NC


def run(inputs: dict, trace: bool = False):
    """Shard inputs over 8 cores, run the SPMD kernel, gather the output."""
    bf = ml_dtypes.bfloat16
    inp = np.asarray(inputs["input"], np.float32).reshape(8, C, HW)
    ctx = np.asarray(inputs["context"], np.float32).reshape(8, E, L)
    proj_in_w = np.asarray(inputs["proj_in_w"], np.float32)
    wq_w = np.asarray(inputs["wq_w"], np.float32)
    wk_w = np.asarray(inputs["wk_w"], np.float32)
    wv_w = np.asarray(inputs["wv_w"], np.float32)
    proj_out_w = np.asarray(inputs["proj_out_w"], np.float32)

    def fp8(x):
        return np.clip(x, -240.0, 240.0).astype(NP_FP8)

    scale = float(E) ** -0.5
    wq_eff = (proj_in_w.T @ wq_w) * scale         # [C, E]
    h_w = SH * (wk_w @ wq_eff.T)                  # [E, C]
    wo_full = proj_out_w.T                        # [C+E, C]
    w_vo = SV * (wv_w @ wo_full[C:])              # [E, C]
    wo_a = wo_full[:C]                            # [C, C]

    # partition-major retiles: one contiguous-burst DMA per tensor/block
    h8 = fp8(h_w.reshape(ET, P, C).transpose(1, 0, 2))
    wv8 = fp8(w_vo.reshape(ET, P, C).transpose(1, 0, 2))
    wo16 = np.ascontiguousarray(
        wo_a.reshape(CT_T, P, C).transpose(1, 0, 2)).astype(bf)
    ct8 = fp8(ctx.reshape(8, ET, P, L).transpose(0, 2, 1, 3))
    a_t = inp.reshape(8, CT_T, P, NBLK, B).transpose(0, 3, 2, 1, 4)
    a16 = np.ascontiguousarray(a_t).astype(bf)    # [8, blk, p, ct, b]
    a8 = fp8(a_t)

    in_maps = [
        {
            "a": a16[i],
            "a8": a8[i],
            "ct": ct8[i],
            "wq": h8,
            "wv": wv8,
            "wo": wo16,
        }
        for i in range(8)
    ]

    nc = _get_nc()
    res = run_bass_kernel_spmd(nc, in_maps, core_ids=list(range(8)),
                               trace=trace)
    out = np.stack([res.results[i]["out"] for i in range(8)])
    # [8, blk, p, ct, b] -> [8, C, 64, 64]
    out = out.astype(np.float32).transpose(0, 3, 2, 1, 4).reshape(8, C, 64, 64)
    return np.ascontiguousarray(out), res


def kernel(**inputs) -> np.ndarray:
    out, _ = run(inputs, trace=False)
    return out
